# revision 14
# baseline (speedup 1.0000x reference)
"""3-layer GAT (single head) on Trainium2, 8 NeuronCores.

Strategy
--------
Nodes are sharded across the 8 cores (6250 nodes each).  Per layer:
  1. dense phase (sharded): h_ext = x_shard @ [W | W@al | W@ar | 0 | 0]
     -> per-node table row [h(128) | el | er | pad2] f32 (528B).
  2. AllGather the table shards -> full 50000-row table in每 core's DRAM.
  3. edge phase (edges sharded by dst, sorted by dst block):
     - indirect-DMA gather of table rows by src (h + el in one 528B row)
     - indirect-DMA gather of er scalars by dst (element_offset=129)
     - per 128-edge tile: one-hot dst mask built on DVE (iota == dstlocal),
       msg = exp(leakyrelu(el+er)) * h on ACT, segment-sum via PE matmul
       (mask^T @ [msg | ex]) accumulated in PSUM per 128-dst block.
     - block epilogue: out = psum[:, :128] / psum[:,128] + b (+relu for L1,2)

The kernel is traced/compiled at call time from the actual edge indices
(tile counts are data-dependent but identical across cores - SPMD).
"""

import math
import numpy as np

import concourse.bacc as bacc
import concourse.bass as bass
import concourse.mybir as mybir
import concourse.tile as tile
from concourse.bass_utils import run_bass_kernel_spmd

P = 128
N_NODES = 50000
N_EDGES = 625000
FEAT = 128
ROW = 132          # h(128), el(128), er(129), pad(130,131)
NCORES = 8
NPC = N_NODES // NCORES          # 6250 nodes per core
NBLK = math.ceil(NPC / P)        # 49 dst blocks per core (48 full + 106)
NPAD = NBLK * P                  # 6272
NEG_SLOPE = 0.2
GROUP_COL_CAP = 56               # max gather columns per indirect-DMA call
IDX_SCALE = 1                    # ROW if HW ignores dynamic-AP coef, else 1

f32 = mybir.dt.float32
i32 = mybir.dt.int32


def _set_sizes(n_nodes, n_edges, ncores=NCORES, feat=FEAT):
    """Test helper: reconfigure module-level sizes (small-scale sim runs)."""
    global N_NODES, N_EDGES, FEAT, ROW, NCORES, NPC, NBLK, NPAD
    N_NODES, N_EDGES, FEAT, NCORES = n_nodes, n_edges, feat, ncores
    ROW = feat + 4
    NPC = N_NODES // NCORES
    NBLK = math.ceil(NPC / P)
    NPAD = NBLK * P
    _CACHE.clear()


# --------------------------------------------------------------------------
# host-side preprocessing
# --------------------------------------------------------------------------

def build_edge_meta(src, dst):
    """Partition edges by dst core/block, sort by src, pack gather indices.

    Returns (T_blk, coloff, TOT, groups, per_core) where per_core[c] is a dict
    with srcI/dstI/dstL arrays of shape [P, TOT].
    """
    src = np.asarray(src, dtype=np.int64)
    dst = np.asarray(dst, dtype=np.int64)

    core = dst // NPC
    loc = dst % NPC
    blk = loc // P
    dstloc = loc % P

    # edges grouped per (core, blk), sorted by src within the group
    order = np.lexsort((src, blk, core))
    s_src, s_core, s_blk, s_dstloc = (
        src[order], core[order], blk[order], dstloc[order])

    counts = np.zeros((NCORES, NBLK), dtype=np.int64)
    np.add.at(counts, (s_core, s_blk), 1)

    # uniform (across cores) tile count per block position
    T_blk = np.maximum(1, np.ceil(counts.max(axis=0) / P).astype(np.int64))
    coloff = np.concatenate([[0], np.cumsum(T_blk)])
    TOT = int(coloff[-1])

    # group consecutive blocks for one indirect-DMA call each
    groups = []  # (blk_start, blk_end, col_start, col_end)
    b0 = 0
    while b0 < NBLK:
        b1 = b0 + 1
        while b1 < NBLK and coloff[b1 + 1] - coloff[b0] <= GROUP_COL_CAP:
            b1 += 1
        groups.append((b0, b1, int(coloff[b0]), int(coloff[b1])))
        b0 = b1

    bounds = np.zeros((NCORES, NBLK + 1), dtype=np.int64)
    starts = np.searchsorted(s_core * NBLK + s_blk,
                             np.arange(NCORES * NBLK + 1))
    # starts[i] = first edge index with core*NBLK+blk >= i
    per_core = []
    for c in range(NCORES):
        srcI = np.zeros((P, TOT), dtype=np.int32)
        dstI = np.zeros((P, TOT), dtype=np.int32)
        dstL = np.full((P, TOT), -1.0, dtype=np.float32)
        for b in range(NBLK):
            i0 = starts[c * NBLK + b]
            i1 = starts[c * NBLK + b + 1]
            k = i1 - i0
            Tb = int(T_blk[b])
            cap = Tb * P
            assert k <= cap
            e_src = s_src[i0:i1]
            e_dl = s_dstloc[i0:i1]
            # pad by repeating the last edge's src (page-hot), dstloc = -1
            pad = cap - k
            if k == 0:
                p_src = np.zeros(cap, dtype=np.int64)
                p_dl = np.full(cap, -1.0)
                p_dst = np.full(cap, c * NPC + b * P, dtype=np.int64)
            else:
                p_src = np.concatenate([e_src, np.full(pad, e_src[-1])])
                p_dl = np.concatenate([e_dl.astype(np.float64),
                                       np.full(pad, -1.0)])
                p_dst = np.concatenate(
                    [c * NPC + b * P + e_dl,
                     np.full(pad, c * NPC + b * P, dtype=np.int64)])
            # edge j -> partition j % P, column j // P (one [P,1] gather/tile)
            c0 = int(coloff[b])
            srcI[:, c0:c0 + Tb] = p_src.reshape(Tb, P).T
            dstI[:, c0:c0 + Tb] = p_dst.reshape(Tb, P).T
            dstL[:, c0:c0 + Tb] = p_dl.reshape(Tb, P).T
        per_core.append({"srcI": srcI, "dstI": dstI, "dstL": dstL})
    return T_blk, coloff, TOT, groups, per_core


def build_weights_ext(W, al, ar):
    """[F, ROW] = [W | W@al | W@ar | 0 | 0]"""
    out = np.zeros((FEAT, ROW), dtype=np.float32)
    out[:, :FEAT] = W
    out[:, FEAT] = W @ al
    out[:, FEAT + 1] = W @ ar
    return out


# --------------------------------------------------------------------------
# device program
# --------------------------------------------------------------------------

DEBUG_LAYER = None  # set to 0/1/2 to dump that layer's table + edge tensors


def build_program(T_blk, coloff, TOT, groups):
    nc = bacc.Bacc("TRN2", target_bir_lowering=False, debug=False,
                   num_devices=NCORES)

    x_pad = nc.dram_tensor("x_pad", [NPAD, FEAT], f32, kind="ExternalInput").ap()
    wext = nc.dram_tensor("wext", [3, FEAT, ROW], f32, kind="ExternalInput").ap()
    bbias = nc.dram_tensor("bbias", [3, P, FEAT], f32, kind="ExternalInput").ap()
    srcI = nc.dram_tensor("srcI", [P, TOT], i32, kind="ExternalInput").ap()
    dstI = nc.dram_tensor("dstI", [P, TOT], i32, kind="ExternalInput").ap()
    dstL = nc.dram_tensor("dstL", [P, TOT], f32, kind="ExternalInput").ap()
    iota_in = nc.dram_tensor("iota_in", [P, P], f32, kind="ExternalInput").ap()
    ident_in = nc.dram_tensor("ident_in", [P, P], f32, kind="ExternalInput").ap()
    out_sh = nc.dram_tensor("out_shard", [NPC, FEAT], f32,
                            kind="ExternalOutput").ap()

    table_shard = nc.dram_tensor("table_shard", [NPC, ROW], f32,
                                 kind="Internal").ap()
    table_full = nc.dram_tensor("table_full", [N_NODES, ROW], f32,
                                kind="Internal", addr_space="Shared").ap()
    x_cur = nc.dram_tensor("x_cur", [NPAD, FEAT], f32, kind="Internal").ap()

    rg = [list(range(NCORES))]
    blk_rows = [P] * (NBLK - 1) + [NPC - P * (NBLK - 1)]

    dbg_table = dbg_G = dbg_ex = None
    if DEBUG_LAYER is not None:
        dbg_table = nc.dram_tensor("dbg_table", [N_NODES, ROW], f32,
                                   kind="ExternalOutput").ap()
        dbg_G = nc.dram_tensor("dbg_G", [P, TOT, ROW], f32,
                               kind="ExternalOutput").ap()
        dbg_ex = nc.dram_tensor("dbg_ex", [P, TOT], f32,
                                kind="ExternalOutput").ap()

    with tile.TileContext(nc, num_cores=NCORES) as tc:
        with (
            tc.tile_pool(name="const", bufs=1) as cpool,
            tc.tile_pool(name="gath", bufs=2) as gpool,
            tc.tile_pool(name="er", bufs=2) as epool,
            tc.tile_pool(name="mask", bufs=4) as mpool,
            tc.tile_pool(name="msg", bufs=4) as msgpool,
            tc.tile_pool(name="small", bufs=4) as spool,
            tc.tile_pool(name="outb", bufs=3) as opool,
            tc.tile_pool(name="dense", bufs=3) as dpool,
            tc.tile_pool(name="psum", bufs=2, space="PSUM") as pspool,
            tc.tile_pool(name="psblk", bufs=2, space="PSUM") as psblk_pool,
        ):
            # ---- persistent SBUF state ----
            srcI_sb = cpool.tile([P, TOT], i32, name="srcI_sb")
            dstI_sb = cpool.tile([P, TOT], i32, name="dstI_sb")
            dstL_sb = cpool.tile([P, TOT], f32, name="dstL_sb")
            iota_sb = cpool.tile([P, P], f32, name="iota_sb")
            ident_sb = cpool.tile([P, P], f32, name="ident_sb")
            wext_sb = cpool.tile([FEAT, ROW], f32, name="wext_sb")
            bb_sb = cpool.tile([P, FEAT], f32, name="bb_sb")
            zero_sb = cpool.tile([P, FEAT], f32, name="zero_sb")

            nc.sync.dma_start(out=srcI_sb[:], in_=srcI)
            nc.sync.dma_start(out=dstI_sb[:], in_=dstI)
            nc.sync.dma_start(out=dstL_sb[:], in_=dstL)
            nc.sync.dma_start(out=iota_sb[:], in_=iota_in)
            nc.sync.dma_start(out=ident_sb[:], in_=ident_in)
            nc.vector.memset(zero_sb[:], 0.0)
            # zero the padding rows of x_cur once
            if NPAD > NPC:
                nc.sync.dma_start(out=x_cur[NPC:NPAD, :],
                                  in_=zero_sb[:NPAD - NPC, :])

            for layer in range(3):
                x_src = x_pad if layer == 0 else x_cur
                nc.sync.dma_start(out=wext_sb[:], in_=wext[layer])
                nc.sync.dma_start(out=bb_sb[:], in_=bbias[layer])

                # ---- dense phase: table_shard = x_shard @ Wext ----
                for i in range(NBLK):
                    sb_x = dpool.tile([P, FEAT], f32, tag="sb_x")
                    nc.sync.dma_start(out=sb_x[:],
                                      in_=x_src[i * P:(i + 1) * P, :])
                    ps_xT = pspool.tile([P, P], f32, tag="ps_xT")
                    nc.tensor.transpose(out=ps_xT[:], in_=sb_x[:],
                                        identity=ident_sb[:])
                    sb_xT = dpool.tile([P, P], f32, tag="sb_xT")
                    nc.vector.tensor_copy(out=sb_xT[:], in_=ps_xT[:])
                    ps_h = pspool.tile([P, ROW], f32, tag="ps_h")
                    nc.tensor.matmul(out=ps_h[:], lhsT=sb_xT[:],
                                     rhs=wext_sb[:], start=True, stop=True)
                    sb_row = dpool.tile([P, ROW], f32, tag="sb_row")
                    nc.scalar.copy(out=sb_row[:], in_=ps_h[:])
                    r = blk_rows[i]
                    nc.sync.dma_start(
                        out=table_shard[i * P:i * P + r, :],
                        in_=sb_row[:r, :])

                # ---- all-gather the table ----
                nc.gpsimd.collective_compute(
                    "AllGather", mybir.AluOpType.bypass,
                    replica_groups=rg,
                    ins=[table_shard], outs=[table_full])

                if DEBUG_LAYER == layer:
                    for i in range(N_NODES // P):
                        tt = dpool.tile([P, ROW], f32, tag="dbg_tt")
                        nc.sync.dma_start(out=tt[:],
                                          in_=table_full[i * P:(i + 1) * P, :])
                        nc.sync.dma_start(out=dbg_table[i * P:(i + 1) * P, :],
                                          in_=tt[:])

                # ---- edge phase: per-tile [P,1] indirect gathers ----
                for b in range(NBLK):
                    Tb = int(T_blk[b])
                    cb = int(coloff[b])
                    ps_blk = psblk_pool.tile([P, FEAT + 1], f32,
                                             tag="ps_blk")
                    for t in range(Tb):
                        c = cb + t
                        G = gpool.tile([P, ROW], f32, tag="G")
                        nc.gpsimd.indirect_dma_start(
                            out=G[:], out_offset=None,
                            in_=table_full,
                            in_offset=bass.IndirectOffsetOnAxis(
                                ap=srcI_sb[:, c:c + 1], axis=0))
                        D = epool.tile([P, ROW], f32, tag="D")
                        nc.gpsimd.indirect_dma_start(
                            out=D[:], out_offset=None,
                            in_=table_full,
                            in_offset=bass.IndirectOffsetOnAxis(
                                ap=dstI_sb[:, c:c + 1], axis=0))
                        if DEBUG_LAYER == layer:
                            nc.sync.dma_start(out=dbg_G[:, c:c + 1, :],
                                              in_=G[:, None, :])
                        t_att = spool.tile([P, 1], f32, tag="t_att")
                        nc.vector.tensor_tensor(
                            out=t_att[:],
                            in0=G[:, FEAT:FEAT + 1],
                            in1=D[:, FEAT + 1:FEAT + 2],
                            op=mybir.AluOpType.add)
                        t_s = spool.tile([P, 1], f32, tag="t_s")
                        nc.vector.tensor_scalar_mul(t_s[:], t_att[:],
                                                    NEG_SLOPE)
                        t_lr = spool.tile([P, 1], f32, tag="t_lr")
                        nc.vector.tensor_tensor(out=t_lr[:], in0=t_att[:],
                                                in1=t_s[:],
                                                op=mybir.AluOpType.max)
                        ex = spool.tile([P, 1], f32, tag="ex")
                        nc.scalar.activation(ex[:], t_lr[:],
                                             mybir.ActivationFunctionType.Exp)
                        if DEBUG_LAYER == layer:
                            nc.sync.dma_start(out=dbg_ex[:, c:c + 1],
                                              in_=ex[:])
                        mask = mpool.tile([P, P], f32, tag="mask")
                        nc.vector.tensor_tensor(
                            out=mask[:],
                            in0=iota_sb[:],
                            in1=dstL_sb[:, c:c + 1].to_broadcast([P, P]),
                            op=mybir.AluOpType.is_equal)
                        msg = msgpool.tile([P, FEAT + 1], f32, tag="msg")
                        nc.scalar.activation(
                            msg[:, 0:FEAT], G[:, 0:FEAT],
                            mybir.ActivationFunctionType.Copy,
                            scale=ex[:])
                        nc.scalar.copy(out=msg[:, FEAT:FEAT + 1], in_=ex[:])
                        nc.tensor.matmul(
                            out=ps_blk[:], lhsT=mask[:], rhs=msg[:],
                            start=(t == 0), stop=(t == Tb - 1))
                    if True:

                        den = spool.tile([P, 1], f32, tag="den")
                        nc.vector.tensor_scalar_add(
                            den[:], ps_blk[:, FEAT:FEAT + 1], 1e-30)
                        rec = spool.tile([P, 1], f32, tag="rec")
                        nc.vector.reciprocal(rec[:], den[:])
                        o1 = opool.tile([P, FEAT], f32, tag="o1")
                        nc.scalar.activation(
                            o1[:], ps_blk[:, 0:FEAT],
                            mybir.ActivationFunctionType.Copy, scale=rec[:])
                        o2 = opool.tile([P, FEAT], f32, tag="o2")
                        nc.vector.tensor_tensor(out=o2[:], in0=o1[:],
                                                in1=bb_sb[:],
                                                op=mybir.AluOpType.add)
                        r = blk_rows[b]
                        if layer < 2:
                            o3 = opool.tile([P, FEAT], f32, tag="o3")
                            nc.vector.tensor_scalar_max(o3[:], o2[:], 0.0)
                            nc.sync.dma_start(
                                out=x_cur[b * P:b * P + r, :], in_=o3[:r, :])
                        else:
                            nc.sync.dma_start(
                                out=out_sh[b * P:b * P + r, :], in_=o2[:r, :])

    nc.compile()
    return nc


# --------------------------------------------------------------------------
# entry point
# --------------------------------------------------------------------------

_CACHE = {}


def _prepare(src, dst):
    key = (src.tobytes()[:64], dst.tobytes()[:64], len(src))
    if key not in _CACHE:
        T_blk, coloff, TOT, groups, per_core = build_edge_meta(src, dst)
        nc = build_program(T_blk, coloff, TOT, groups)
        _CACHE[key] = (nc, per_core)
    return _CACHE[key]


def kernel(x, src, dst, W1, al1, ar1, b1, W2, al2, ar2, b2, W3, al3, ar3, b3,
           trace=False):
    x = np.asarray(x, dtype=np.float32)
    src = np.asarray(src, dtype=np.int32)
    dst = np.asarray(dst, dtype=np.int32)

    nc, per_core = _prepare(src, dst)

    wext = np.stack([
        build_weights_ext(np.asarray(W, np.float32), np.asarray(al, np.float32),
                          np.asarray(ar, np.float32))
        for W, al, ar in ((W1, al1, ar1), (W2, al2, ar2), (W3, al3, ar3))])
    bbias = np.stack([
        np.broadcast_to(np.asarray(b, np.float32), (P, FEAT)).copy()
        for b in (b1, b2, b3)])
    iota = np.broadcast_to(np.arange(P, dtype=np.float32), (P, P)).copy()
    ident = np.eye(P, dtype=np.float32)

    in_maps = []
    for c in range(NCORES):
        xs = np.zeros((NPAD, FEAT), dtype=np.float32)
        xs[:NPC] = x[c * NPC:(c + 1) * NPC]
        in_maps.append({
            "x_pad": xs,
            "wext": wext,
            "bbias": bbias,
            "srcI": per_core[c]["srcI"],
            "dstI": per_core[c]["dstI"],
            "dstL": per_core[c]["dstL"],
            "iota_in": iota,
            "ident_in": ident,
        })

    res = run_bass_kernel_spmd(nc, in_maps, core_ids=list(range(NCORES)),
                               trace=trace)
    out = np.concatenate([res.results[c]["out_shard"] for c in range(NCORES)],
                         axis=0)
    kernel.last_results = res
    return out
